# revision 1
# baseline (speedup 1.0000x reference)
"""Self-contained SwinTransformerBlockV2 kernel.

Contract: kernel(**inputs) takes the FULL unsharded inputs (see shapes
below) and returns the FULL (8, 256, 256, 64) float32 output.

Strategy: pure data parallel over batch B=8 — one batch element per
NeuronCore when 8 neuron devices are visible (jax pmap, params
replicated, windows independent so no cross-device comms). Falls back
to single-device CPU execution if the neuron path is unavailable.

Hardcoded problem constants (per spec): B=8, H=W=256, C=64, 4 heads,
window 4x4, shift 8x8.
"""

import math

import numpy as np

DIM = 64
NUM_HEADS = 4
HEAD_DIM = DIM // NUM_HEADS
WS = (4, 4)
SS = (8, 8)


def _rel_position_index():
    ch, cw = np.meshgrid(np.arange(WS[0]), np.arange(WS[1]), indexing="ij")
    coords = np.stack([ch, cw]).reshape(2, -1)
    rel = (coords[:, :, None] - coords[:, None, :]).transpose(1, 2, 0)
    rel[:, :, 0] += WS[0] - 1
    rel[:, :, 1] += WS[1] - 1
    rel[:, :, 0] *= 2 * WS[1] - 1
    return rel.sum(-1).reshape(-1)


def _rel_coords_table():
    h = np.arange(-(WS[0] - 1), WS[0], dtype=np.float32)
    w = np.arange(-(WS[1] - 1), WS[1], dtype=np.float32)
    t = np.stack(np.meshgrid(h, w, indexing="ij"), axis=-1)
    t[..., 0] /= WS[0] - 1
    t[..., 1] /= WS[1] - 1
    t *= 8.0
    t = np.sign(t) * np.log2(np.abs(t) + 1.0) / 3.0
    return t.reshape(-1, 2).astype(np.float32)


def _attn_mask(H, W):
    m = np.zeros((H, W), np.float32)
    hs = (slice(0, -WS[0]), slice(-WS[0], -SS[0]), slice(-SS[0], None))
    ws_ = (slice(0, -WS[1]), slice(-WS[1], -SS[1]), slice(-SS[1], None))
    cnt = 0
    for h in hs:
        for w in ws_:
            m[h, w] = cnt
            cnt += 1
    nWh, nWw = H // WS[0], W // WS[1]
    m = (
        m.reshape(nWh, WS[0], nWw, WS[1])
        .transpose(0, 2, 1, 3)
        .reshape(nWh * nWw, WS[0] * WS[1])
    )
    d = m[:, None, :] - m[:, :, None]
    return np.where(d != 0, -100.0, 0.0).astype(np.float32)


def _build_block_fn(jnp, jax):
    rel_idx = np.asarray(_rel_position_index())
    rel_tab = _rel_coords_table()

    def _layernorm(x, g, b):
        m = x.mean(-1, keepdims=True)
        v = x.var(-1, keepdims=True)
        return (x - m) / jnp.sqrt(v + 1e-5) * g + b

    def block(
        x,
        qkv_w,
        qkv_b,
        proj_w,
        proj_b,
        logit_scale,
        cpb_w1,
        cpb_b1,
        cpb_w2,
        norm1_g,
        norm1_b,
        norm2_g,
        norm2_b,
        fc1_w,
        fc1_b,
        fc2_w,
        fc2_b,
    ):
        B, H, W, C = x.shape
        nWh, nWw = H // WS[0], W // WS[1]
        nW, N = nWh * nWw, WS[0] * WS[1]

        xs = jnp.roll(x, (-SS[0], -SS[1]), axis=(1, 2))
        xw = (
            xs.reshape(B, nWh, WS[0], nWw, WS[1], C)
            .transpose(0, 1, 3, 2, 4, 5)
            .reshape(B * nW, N, C)
        )

        qkv_b2 = qkv_b.at[C : 2 * C].set(0.0)
        qkv = xw @ qkv_w.T + qkv_b2
        qkv = qkv.reshape(B * nW, N, 3, NUM_HEADS, HEAD_DIM).transpose(2, 0, 3, 1, 4)
        q, k, v = qkv[0], qkv[1], qkv[2]

        qn = q / jnp.maximum(jnp.linalg.norm(q, axis=-1, keepdims=True), 1e-12)
        kn = k / jnp.maximum(jnp.linalg.norm(k, axis=-1, keepdims=True), 1e-12)
        attn = jnp.einsum("whnd,whmd->whnm", qn, kn)
        scale = jnp.exp(jnp.minimum(logit_scale, math.log(100.0)))
        attn = attn * scale[None]

        table = (
            jax.nn.relu(jnp.asarray(rel_tab) @ cpb_w1.T + cpb_b1) @ cpb_w2.T
        )
        bias = table[jnp.asarray(rel_idx)].reshape(N, N, NUM_HEADS).transpose(2, 0, 1)
        bias = 16.0 * jax.nn.sigmoid(bias)
        attn = attn + bias[None]

        mask = jnp.asarray(_attn_mask(H, W))
        attn = attn.reshape(B, nW, NUM_HEADS, N, N) + mask[None, :, None]
        attn = jax.nn.softmax(attn.reshape(B * nW, NUM_HEADS, N, N), axis=-1)

        out = (
            jnp.einsum("whnm,whmd->whnd", attn, v)
            .transpose(0, 2, 1, 3)
            .reshape(B * nW, N, C)
        )
        out = out @ proj_w.T + proj_b

        out = (
            out.reshape(B, nWh, nWw, WS[0], WS[1], C)
            .transpose(0, 1, 3, 2, 4, 5)
            .reshape(B, H, W, C)
        )
        a = jnp.roll(out, (SS[0], SS[1]), axis=(1, 2))

        x = x + _layernorm(a, norm1_g, norm1_b)
        h = (
            jax.nn.gelu(x @ fc1_w.T + fc1_b, approximate=False) @ fc2_w.T
            + fc2_b
        )
        x = x + _layernorm(h, norm2_g, norm2_b)
        return x

    return block


_PARAM_NAMES = (
    "qkv_w",
    "qkv_b",
    "proj_w",
    "proj_b",
    "logit_scale",
    "cpb_w1",
    "cpb_b1",
    "cpb_w2",
    "norm1_g",
    "norm1_b",
    "norm2_g",
    "norm2_b",
    "fc1_w",
    "fc1_b",
    "fc2_w",
    "fc2_b",
)


def _run_neuron(jax, jnp, block, x, params):
    """Data-parallel over batch across 8 NeuronCores via pmap."""
    devs = [d for d in jax.devices() if d.platform != "cpu"][:8]
    if len(devs) < 8:
        raise RuntimeError(f"need 8 accel devices, have {len(devs)}")
    fn = jax.pmap(
        block,
        axis_name="b",
        in_axes=(0,) + (None,) * len(_PARAM_NAMES),
        devices=devs,
    )
    xs = x.reshape(8, 1, 256, 256, 64)
    out = fn(xs, *params)
    return np.asarray(out).reshape(8, 256, 256, 64)


def kernel(**inputs):
    import jax
    import jax.numpy as jnp

    x = np.asarray(inputs["x"], dtype=np.float32)
    params = tuple(np.asarray(inputs[n]) for n in _PARAM_NAMES)
    block = _build_block_fn(jnp, jax)

    try:
        out = _run_neuron(jax, jnp, block, x, params)
    except Exception:
        cpu = jax.devices("cpu")[0]
        fn = jax.jit(block, device=cpu)
        out = np.asarray(
            fn(*jax.device_put((x,) + params, cpu))
        )
    return out.astype(np.float32)


if __name__ == "__main__":
    pass
